# revision 19
# baseline (speedup 1.0000x reference)
"""Causal self-attention on 8 TRN2 NeuronCores (Bass/Tile, SPMD).

Problem: B=4, T=2048, C=1024, H=16, D=64, fp32 in/out.

Sharding: core i = (batch b=i//2, parity p=i%2). Each core computes all 16
heads for 8 interleaved 128-wide q-tiles of its batch, sorted ascending by
causal prefix length so both parities pad to the identical extent multiset
EXT = {2,4,...,16} k-tiles -> every core runs the same instruction stream
(SPMD); causality handled by host-supplied mask data. K/V are computed for
the full sequence on both cores of a batch.

All matmul data is bf16 (~5e-3 end-to-end rel err vs fp64): DMA stays
small and every matmul streams 512-col moving operands (the PSUM-bank
cap), with stationary reuse across both halves.

Schedule (single interleaved instruction stream; engines overlap):
  1. Q^T (own q, slot order) -> QT[j] SBUF.
  2. K^T slab 0 (k-tiles 0-7) and V_aug tiles 0-7.
  3. Attention pass A (q-slots 0-3, k-tiles 0-7) per head-pair j,
     interleaved with production of K^T slab 1 and V_aug 8-15 so the PE
     stays dense while exp runs on ScalarE (lag-1 PV software pipeline).
  4. Attention pass B (q-slots 4-7, k-tiles 0-15) interleaved with the
     output projection of the finished slots 0-3.
  5. Projection tail (slots 4-7).
Attention per (j, kt): S^T for both heads into one PSUM tile [128,2x512],
one batched exp (ScalarE) -> bf16 P^T, one mask multiply (DVE), PV
accumulation into per-head O^T psum [65,512] (row 64 = softmax sums l).
Normalize once per head/pass: reciprocal (DVE) + partition broadcast
(GpSimd) + multiply (DVE), writing O^T into the dead Q^T columns.
bias_eff = bproj + bv @ Wproj folds the V-bias exactly (softmax rows sum
to 1).
"""
import os
import numpy as np
import ml_dtypes

import concourse.bacc as bacc
import concourse.mybir as mybir
import concourse.tile as tile
from concourse.bass_utils import run_bass_kernel_spmd

B, T, C, H, D = 4, 2048, 1024, 16, 64
NQT = 8                         # own q-tiles (128 wide) per core
EXT = [2, 4, 6, 8, 10, 12, 14, 16]   # padded k-extent (128-tiles) per slot
# global q-tile ids per parity, slot order (ascending exact extent)
OWN = [[0, 3, 4, 7, 8, 11, 12, 15], [1, 2, 5, 6, 9, 10, 13, 14]]
F32 = mybir.dt.float32
BF = mybir.dt.bfloat16
VA_W = H * (D + 1)              # 1040: V_aug cols = 16 heads x (64 + ones)

_cache = {}


def _build():
    nc = bacc.Bacc("TRN2", target_bir_lowering=False, debug=False,
                   enable_asserts=False, num_devices=8)
    def din(name, shape, dt=BF):
        return nc.dram_tensor(name, list(shape), dt, kind="ExternalInput").ap()

    xt_d = din("xt", (C, T))            # x[b].T, bf16
    xq_d = din("xq", (C, NQT * 128))    # own q columns of xt, slot order
    wq_d = din("wq", (C, C))            # pre-scaled by 1/8
    wk_d = din("wk", (C, C))
    wv_d = din("wv", (C, C))
    wp_d = din("wp", (C, C))
    bq_d = din("bq", (8, 128, 1), F32)
    bk_d = din("bk", (8, 128, 1), F32)
    bpeb_d = din("bpeb", (128, C), F32) # bproj_eff broadcast to 128 partitions
    mk_d = din("masks", (16, 128, 128)) # per-kt causal mask tile
    y_d = nc.dram_tensor("y", [NQT * 128, C], F32, kind="ExternalOutput").ap()

    bypass = mybir.AluOpType.bypass
    mult = mybir.AluOpType.mult
    add = mybir.AluOpType.add
    EXP = mybir.ActivationFunctionType.Exp

    with tile.TileContext(nc) as tc:
        # ---------------- persistent SBUF tensors ------------------------
        ktp = tc.alloc_tile_pool(name="ktp", bufs=1)
        KT = [ktp.tile([128, T], BF, name=f"kt{j}", tag=f"kt{j}") for j in range(8)]
        QT = [ktp.tile([128, 1024], BF, name=f"qt{j}", tag=f"qt{j}") for j in range(8)]
        VA = [ktp.tile([128, VA_W], BF, name=f"va{g}", tag=f"va{g}") for g in range(16)]

        # ---------------- block A: Q^T -----------------------------------
        with tc.tile_pool(name="pa", bufs=1) as wqp, \
             tc.tile_pool(name="pax", bufs=1) as xqp, \
             tc.tile_pool(name="pap", bufs=2, space="PSUM") as psA:
            wqc = [wqp.tile([128, C], BF, name=f"wqc{c}", tag=f"wqc{c}") for c in range(8)]
            bqs = [wqp.tile([128, 1], F32, name=f"bqs{j}", tag=f"bqs{j}") for j in range(8)]
            xqc = [xqp.tile([128, 1024], BF, name=f"xqc{c}", tag=f"xqc{c}") for c in range(8)]
            for c in range(8):
                nc.sync.dma_start(out=wqc[c][:], in_=wq_d[128*c:128*(c+1), :])
                nc.sync.dma_start(out=bqs[c][:], in_=bq_d[c])
                nc.sync.dma_start(out=xqc[c][:], in_=xq_d[128*c:128*(c+1), :])
            for j in range(8):
                pq = [psA.tile([128, 512], F32, name=f"pqh{h}", tag=f"pqh{h}")
                      for h in range(2)]
                for c in range(8):
                    for h in range(2):
                        nc.tensor.matmul(out=pq[h][:],
                                         lhsT=wqc[c][:, 128*j:128*(j+1)],
                                         rhs=xqc[c][:, 512*h:512*(h+1)],
                                         start=(c == 0), stop=(c == 7))
                for h in range(2):
                    nc.vector.tensor_scalar_add(out=QT[j][:, 512*h:512*(h+1)],
                                                in0=pq[h][:], scalar1=bqs[j][:])

        # ------------- main scope: K/V production + attention + proj -----
        with tc.tile_pool(name="pw", bufs=1) as wpool, \
             tc.tile_pool(name="px", bufs=1) as xsp, \
             tc.tile_pool(name="mkp", bufs=1) as mkp, \
             tc.tile_pool(name="ptp", bufs=4) as ptp, \
             tc.tile_pool(name="smp", bufs=1) as smp, \
             tc.tile_pool(name="p3y", bufs=2) as yp, \
             tc.tile_pool(name="psP", bufs=1, space="PSUM") as psP, \
             tc.tile_pool(name="psS", bufs=2, space="PSUM") as psS, \
             tc.tile_pool(name="psO", bufs=1, space="PSUM") as psO:
            wkc = [wpool.tile([128, C], BF, name=f"wkc{c}", tag=f"wkc{c}") for c in range(8)]
            wvc = [wpool.tile([128, C], BF, name=f"wvc{c}", tag=f"wvc{c}") for c in range(8)]
            wpc = [wpool.tile([128, C], BF, name=f"wpc{c}", tag=f"wpc{c}") for c in range(8)]
            bks = [wpool.tile([128, 1], F32, name=f"bks{j}", tag=f"bks{j}") for j in range(8)]
            bpeb = wpool.tile([128, C], F32, name="bpeb", tag="bpeb")
            ones16 = wpool.tile([128, H], F32, name="ones16", tag="ones16")
            xts = [[xsp.tile([128, 1024], BF, name=f"xts{s}{c}", tag=f"xts{s}{c}")
                    for c in range(8)] for s in range(2)]
            MK = []
            for c in range(8):
                nc.sync.dma_start(out=wkc[c][:], in_=wk_d[128*c:128*(c+1), :])
                nc.sync.dma_start(out=bks[c][:], in_=bk_d[c])
            for s in range(2):
                for c in range(8):
                    nc.sync.dma_start(out=xts[s][c][:],
                                      in_=xt_d[128*c:128*(c+1), 1024*s:1024*(s+1)])
            for c in range(8):
                nc.sync.dma_start(out=wvc[c][:], in_=wv_d[128*c:128*(c+1), :])
            for kt in range(16):
                mt = mkp.tile([128, 128], BF, name=f"mk{kt}", tag=f"mk{kt}")
                nc.sync.dma_start(out=mt[:], in_=mk_d[kt])
                MK.append(mt)
            for c in range(8):
                nc.sync.dma_start(out=wpc[c][:], in_=wp_d[128*c:128*(c+1), :])
            nc.sync.dma_start(out=bpeb[:], in_=bpeb_d[:])
            nc.vector.memset(ones16[:], 1.0)
            ones16_3d = ones16[:].unsqueeze(2)
            for g in range(16):
                dst1 = VA[g][:].rearrange("p (h e) -> p h e", e=D+1)[:, :, D:D+1]
                nc.vector.tensor_copy(out=dst1, in_=ones16_3d)

            # -------- production / projection units ----------------------
            def unit_K_half(s, j, h):
                pk = psP.tile([128, 512], F32, name=f"ph{h}", tag=f"ph{h}")
                for c in range(8):
                    nc.tensor.matmul(out=pk[:],
                                     lhsT=wkc[c][:, 128*j:128*(j+1)],
                                     rhs=xts[s][c][:, 512*h:512*(h+1)],
                                     start=(c == 0), stop=(c == 7))
                nc.vector.tensor_scalar_add(
                    out=KT[j][:, 1024*s + 512*h:1024*s + 512*(h+1)],
                    in0=pk[:], scalar1=bks[j][:])

            def unit_V_half(kt, h):
                s, ktl = kt // 8, kt % 8
                pv = psP.tile([128, 512], F32, name=f"ph{h}", tag=f"ph{h}")
                for c in range(8):
                    nc.tensor.matmul(out=pv[:],
                                     lhsT=xts[s][c][:, 128*ktl:128*(ktl+1)],
                                     rhs=wvc[c][:, 512*h:512*(h+1)],
                                     start=(c == 0), stop=(c == 7))
                dst = VA[kt][:, 520*h:520*(h+1)].rearrange(
                    "p (h e) -> p h e", e=D+1)[:, :, 0:D]
                src = pv[:].rearrange("p (h d) -> p h d", d=D)
                nc.vector.tensor_copy(out=dst, in_=src)

            def unit_K(s, j):
                for h in range(2):
                    unit_K_half(s, j, h)

            def unit_V(kt):
                for h in range(2):
                    unit_V_half(kt, h)

            def unit_proj(ti, jc):
                py = psP.tile([128, 512], F32, name=f"ph{jc%2}", tag=f"ph{jc%2}")
                for c in range(8):
                    nc.tensor.matmul(out=py[:],
                                     lhsT=QT[c][:, 128*ti:128*(ti+1)],
                                     rhs=wpc[c][:, 512*jc:512*(jc+1)],
                                     start=(c == 0), stop=(c == 7))
                ysb = yp.tile([128, 512], F32, name="ysb", tag="ysb")
                nc.vector.scalar_tensor_tensor(out=ysb[:], in0=py[:], scalar=0.0,
                                               in1=bpeb[:, 512*jc:512*(jc+1)],
                                               op0=bypass, op1=add)
                nc.sync.dma_start(out=y_d[128*ti:128*(ti+1), 512*jc:512*(jc+1)],
                                  in_=ysb[:])

            # -------- attention passes ------------------------------------
            def emit_pv(j, oa, ob, pend, base):
                pt, kt, lo, cn = pend
                for si in range(cn):
                    s = lo + si
                    for half, (acc, h) in enumerate(((oa, 2*j), (ob, 2*j+1))):
                        # start=True clears the whole PSUM bank: only the
                        # first PV into each [65,512] acc may carry it.
                        # (A merged per-head N=128*cn PV measured *slower*
                        # than these per-slot N=128 matmuls.)
                        nc.tensor.matmul(
                            out=acc[:, 128*(s-base):128*(s-base+1)],
                            lhsT=VA[kt][:, 65*h:65*(h+1)],
                            rhs=pt[:, 512*half + 128*si:512*half + 128*(si+1)],
                            start=(kt == 0 and si == 0),
                            stop=(kt == EXT[s] - 1))

            def emit_norm(j, oa, ob, pas):
                cols = slice(512*pas, 512*(pas+1))
                for half, acc in enumerate((oa, ob)):
                    lsb = smp.tile([1, 512], F32, name="lsb", tag=f"lsb{half}")
                    nc.vector.tensor_copy(out=lsb[:], in_=acc[64:65, :])
                    rsb = smp.tile([1, 512], F32, name="rsb", tag=f"rsb{half}")
                    nc.vector.reciprocal_approx_fast(rsb[:], lsb[:])
                    rbb = smp.tile([64, 512], F32, name="rbb", tag=f"rbb{half}")
                    nc.gpsimd.partition_broadcast(rbb[:], rsb[:])
                    nc.vector.scalar_tensor_tensor(
                        out=QT[j][64*half:64*(half+1), cols], in0=acc[0:64, :],
                        scalar=0.0, in1=rbb[:], op0=bypass, op1=mult)

            def attn_pass(j, pas, fillers=None):
                base = 4 * pas            # first slot of this pass
                kts = range(8) if pas == 0 else range(16)
                oa = psO.tile([65, 512], F32, name="oa", tag="oa")
                ob = psO.tile([65, 512], F32, name="ob", tag="ob")
                pending = []
                fillers = list(fillers or [])
                for kt in kts:
                    lo = max(kt >> 1, base)
                    cn = base + 4 - lo
                    N = 128 * cn
                    qcols = slice(128*lo, 128*lo + N)
                    ss = psS.tile([128, 1024], F32, name="ss", tag="ss")
                    nc.tensor.matmul(out=ss[:, 0:N],
                                     lhsT=KT[j][0:64, 128*kt:128*(kt+1)],
                                     rhs=QT[j][0:64, qcols], tile_position=(0, 0),
                                     start=True, stop=True)
                    nc.tensor.matmul(out=ss[:, 512:512+N],
                                     lhsT=KT[j][64:128, 128*kt:128*(kt+1)],
                                     rhs=QT[j][64:128, qcols], tile_position=(64, 0),
                                     start=True, stop=True)
                    pt = ptp.tile([128, 1024], BF, name="pt", tag="pt")
                    ss3 = ss[:].rearrange("p (two q) -> p two q", two=2)[:, :, 0:N]
                    pt3 = pt[:].rearrange("p (two q) -> p two q", two=2)[:, :, 0:N]
                    nc.scalar.activation(out=pt3, in_=ss3, func=EXP)
                    if (kt >> 1) == lo:
                        # mask the first suffix slot's columns (slot kt//2)
                        for half in range(2):
                            ptm = pt[:, 512*half:512*half + 128]
                            nc.vector.scalar_tensor_tensor(
                                out=ptm, in0=ptm, scalar=0.0, in1=MK[kt][:],
                                op0=bypass, op1=mult)
                    # lag-2 PV pipeline (pt ring of 4 keeps ACT fed)
                    if len(pending) == 2:
                        emit_pv(j, oa, ob, pending.pop(0), base)
                    pending.append((pt, kt, lo, cn))
                    if fillers and kt % 2 == 1:
                        fillers.pop(0)()
                for f in fillers:
                    f()
                for pend in pending:
                    emit_pv(j, oa, ob, pend, base)
                emit_norm(j, oa, ob, pas)

            # -------- schedule --------------------------------------------
            for j in range(8):
                unit_K(0, j)
            for kt in range(8):
                unit_V(kt)
            from functools import partial
            for j in range(8):
                attn_pass(j, 0, [partial(unit_K_half, 1, j, 0),
                                 partial(unit_K_half, 1, j, 1),
                                 partial(unit_V_half, 8 + j, 0),
                                 partial(unit_V_half, 8 + j, 1)])
            for j in range(8):
                attn_pass(j, 1, [partial(unit_proj, j // 2, j % 2)])
            for ti in range(4, 8):
                unit_proj(ti, 0)
                unit_proj(ti, 1)
        ktp.release()

    nc.compile()
    return nc


def _get_nc():
    if "nc" not in _cache:
        _cache["nc"] = _build()
    return _cache["nc"]


def _host_prep(x, Wqkv, bqkv, Wproj, bproj):
    bf16 = ml_dtypes.bfloat16
    x = np.asarray(x, dtype=np.float32)
    Wqkv = np.asarray(Wqkv, dtype=np.float32)
    bqkv = np.asarray(bqkv, dtype=np.float32)
    Wproj = np.ascontiguousarray(np.asarray(Wproj, dtype=np.float32))
    bproj = np.asarray(bproj, dtype=np.float32)

    wq = np.ascontiguousarray(Wqkv[:, :C] * np.float32(0.125)).astype(bf16)
    wk = np.ascontiguousarray(Wqkv[:, C:2*C]).astype(bf16)
    wv = np.ascontiguousarray(Wqkv[:, 2*C:]).astype(bf16)
    wp = Wproj.astype(bf16)
    bq8 = (bqkv[:C] * np.float32(0.125)).reshape(8, 128, 1).copy()
    bk8 = bqkv[C:2*C].reshape(8, 128, 1).copy()
    bv = bqkv[2*C:]
    bpe = (bproj.astype(np.float64) + bv.astype(np.float64) @ Wproj.astype(np.float64)).astype(np.float32)
    bpeb = np.ascontiguousarray(np.broadcast_to(bpe, (128, C)))

    # masks: one [128,128] tile per kt (duplicated across the 2 head halves),
    # applied to P^T rows=k cols=q for slot s = kt//2:
    #   kt == exact-1: diagonal tile; kt < exact-1: all ones; kt >= exact: zeros
    pidx = np.arange(128)[:, None]
    fidx = np.arange(128)[None, :]
    masks = []
    for par in range(2):
        mk = np.zeros((16, 128, 128), dtype=np.float32)
        for kt in range(16):
            s = kt // 2
            g = OWN[par][s]
            exact = g + 1
            if kt < exact - 1:
                m = np.ones((128, 128), dtype=np.float32)
            elif kt == exact - 1:
                m = ((128*kt + pidx) <= (128*g + fidx)).astype(np.float32)
            else:
                m = np.zeros((128, 128), dtype=np.float32)
            mk[kt] = m
        masks.append(mk.astype(bf16))

    in_maps = []
    for core in range(8):
        b, par = core // 2, core % 2
        xt = np.ascontiguousarray(x[b].T).astype(bf16)
        xq = np.ascontiguousarray(
            np.concatenate([xt[:, 128*g:128*(g+1)] for g in OWN[par]], axis=1))
        in_maps.append(dict(xt=xt, xq=xq, wq=wq, wk=wk, wv=wv, wp=wp,
                            bq=bq8, bk=bk8, bpeb=bpeb, masks=masks[par]))
    return in_maps


def kernel(x, Wqkv, bqkv, Wproj, bproj):
    nc = _get_nc()
    in_maps = _host_prep(x, Wqkv, bqkv, Wproj, bproj)
    trace = bool(os.environ.get("BASS_TRACE"))
    res = run_bass_kernel_spmd(nc, in_maps, list(range(8)), trace=trace)
    _cache["last_exec_time_ns"] = res.exec_time_ns
    _cache["last_res"] = res
    out = np.empty((B, T, C), dtype=np.float32)
    for core in range(8):
        b, par = core // 2, core % 2
        y = res.results[core]["y"]
        for s, g in enumerate(OWN[par]):
            out[b, 128*g:128*(g+1)] = y[128*s:128*(s+1)]
    return out
